# revision 2
# baseline (speedup 1.0000x reference)
"""DCL loss kernel for Trainium2 (8 NeuronCores, Bass/Tile).

Math (matches reference):
  centers[i]   = mean of samples with target i           (host, exact)
  dist[i,j]    = ||centers[i] - x[j]||                   (device, bf16 matmul + ACT sqrt)
  d_neg[i]     = mean of dist over valid negatives       (device rowsums + AllReduce,
                                                          pos part subtracted via host input)
  an_mean      = mean_i [ sum_{neg, dist<d_neg} dist / count ]
  ap_mean      = mean of positive dists                  (host, exact)
  out          = ap_mean / an_mean

Device avoids all per-element masking: it computes over ALL columns
  rs[i]  = sum_j dist[i,j]
  C[i]   = #{j : dist[i,j]^2 < d_neg[i]^2}
  M[i]   = sum_j min(dist[i,j], d_neg[i])        (via sqrt(min(d2, d_neg^2)))
and the host removes the positive-pair contributions exactly:
  S_hard = M - d_neg*(N - C) - possum_under ;  C_hard = C - poscnt_under

Sharding: data-parallel over the N sample axis (4096 columns per core);
centers replicated; [4096] rowsum vector all-reduced on device.
"""
import numpy as np
import ml_dtypes

import concourse.bacc as bacc
import concourse.tile as tile
from concourse import mybir
from concourse.bass_utils import run_bass_kernel_spmd

N = 32768
D = 256
NUM_POS = 4
TEMPS = 2
ID = N // TEMPS // NUM_POS  # 4096
CORES = 8
J = N // CORES              # 4096 local samples per core
CHUNKS = ID // 128          # 32 row chunks
GCOLS = 2048                # columns per PSUM group (4 banks)
GROUPS = J // GCOLS         # 2 col groups per chunk
EPS = 1e-6

F32 = mybir.dt.float32
BF16 = mybir.dt.bfloat16

_CACHE = {}


def _build(replicas: int = 1):
    nc = bacc.Bacc("TRN2", target_bir_lowering=False, debug=False,
                   num_devices=CORES)

    a0 = nc.dram_tensor("a0", [128, ID], BF16, kind="ExternalInput")
    a1 = nc.dram_tensor("a1", [128, ID], BF16, kind="ExternalInput")
    b0 = nc.dram_tensor("b0", [128, J], BF16, kind="ExternalInput")
    b1 = nc.dram_tensor("b1", [128, J], BF16, kind="ExternalInput")
    lhs2 = nc.dram_tensor("lhs2", [2, ID], F32, kind="ExternalInput")
    rhs2 = nc.dram_tensor("rhs2", [2, J], F32, kind="ExternalInput")
    possum = nc.dram_tensor("possum", [128, CHUNKS], F32, kind="ExternalInput")
    invn = nc.dram_tensor("invn", [128, CHUNKS], F32, kind="ExternalInput")

    dneg_o = nc.dram_tensor("dneg", [128, CHUNKS], F32, kind="ExternalOutput")
    c_o = nc.dram_tensor("c32", [128, CHUNKS], F32, kind="ExternalOutput")
    m_o = nc.dram_tensor("m32", [128, CHUNKS], F32, kind="ExternalOutput")
    rs_o = nc.dram_tensor("rs32", [128, CHUNKS], F32, kind="ExternalOutput")

    with tile.TileContext(nc) as tc:
        with (
            tc.tile_pool(name="inp", bufs=1) as inp,
            tc.tile_pool(name="acc", bufs=1) as accp,
            tc.tile_pool(name="wrk", bufs=3) as wrk,
            tc.tile_pool(name="ps", bufs=2, space="PSUM") as ps,
            tc.tile_pool(name="dram", bufs=1, space="DRAM") as dram,
        ):
            a0t = inp.tile([128, ID], BF16, tag="a0")
            a1t = inp.tile([128, ID], BF16, tag="a1")
            b0t = inp.tile([128, J], BF16, tag="b0")
            b1t = inp.tile([128, J], BF16, tag="b1")
            l2t = inp.tile([2, ID], F32, tag="l2")
            r2t = inp.tile([2, J], F32, tag="r2")
            pst = inp.tile([128, CHUNKS], F32, tag="pos")
            invt = inp.tile([128, CHUNKS], F32, tag="inv")
            nc.sync.dma_start(a0t[:], a0[:])
            nc.sync.dma_start(a1t[:], a1[:])
            nc.sync.dma_start(b0t[:], b0[:])
            nc.sync.dma_start(b1t[:], b1[:])
            nc.sync.dma_start(l2t[:], lhs2[:])
            nc.sync.dma_start(r2t[:], rhs2[:])
            nc.sync.dma_start(pst[:], possum[:])
            nc.sync.dma_start(invt[:], invn[:])

            for rep in range(replicas):
                # ---- pass 1: rowsums of dist ----
                rsgA = accp.tile([128, CHUNKS], F32, tag=f"rsgA{rep}")
                rsgB = accp.tile([128, CHUNKS], F32, tag=f"rsgB{rep}")
                for r in range(CHUNKS):
                    ra, rb = r * 128, (r + 1) * 128
                    for g in range(GROUPS):
                        p1 = ps.tile([128, GCOLS], F32, tag="pp")
                        for q in range(GCOLS // 512):
                            c0 = g * GCOLS + q * 512
                            qs = slice(q * 512, (q + 1) * 512)
                            nc.tensor.matmul(p1[:, qs], a0t[:, ra:rb],
                                             b0t[:, c0:c0 + 512],
                                             start=True, stop=False)
                            nc.tensor.matmul(p1[:, qs], a1t[:, ra:rb],
                                             b1t[:, c0:c0 + 512],
                                             start=False, stop=False)
                            nc.tensor.matmul(p1[:, qs], l2t[:, ra:rb],
                                             r2t[:, c0:c0 + 512],
                                             start=False, stop=True)
                        dist = wrk.tile([128, GCOLS], BF16, tag="dist")
                        nc.scalar.activation(
                            dist[:], p1[:], mybir.ActivationFunctionType.Sqrt,
                            accum_out=(rsgA if g == 0 else rsgB)[:, r:r + 1])

                # combine slot sums -> rs32; AllReduce; d_neg
                rs32 = accp.tile([128, CHUNKS], F32, tag=f"rs32{rep}")
                nc.vector.tensor_tensor(rs32[:], rsgA[:], rsgB[:],
                                        op=mybir.AluOpType.add)
                arin = dram.tile([128, CHUNKS], F32, tag=f"arin{rep}")
                arout = dram.tile([128, CHUNKS], F32, tag=f"arout{rep}")
                nc.sync.dma_start(arin[:], rs32[:])
                nc.gpsimd.collective_compute(
                    "AllReduce", mybir.AluOpType.add,
                    replica_groups=[list(range(CORES))],
                    ins=[arin.opt()], outs=[arout.opt()],
                )
                rsar = accp.tile([128, CHUNKS], F32, tag=f"rsar{rep}")
                nc.sync.dma_start(rsar[:], arout[:])

                dneg = accp.tile([128, CHUNKS], F32, tag=f"dneg{rep}")
                nc.vector.tensor_tensor(dneg[:], rsar[:], pst[:],
                                        op=mybir.AluOpType.subtract)
                nc.vector.tensor_tensor(dneg[:], dneg[:], invt[:],
                                        op=mybir.AluOpType.mult)
                dsq = accp.tile([128, CHUNKS], F32, tag=f"dsq{rep}")
                nc.vector.tensor_tensor(dsq[:], dneg[:], dneg[:],
                                        op=mybir.AluOpType.mult)

                # ---- pass 2: threshold stats ----
                cgA = accp.tile([128, CHUNKS], F32, tag=f"cgA{rep}")
                cgB = accp.tile([128, CHUNKS], F32, tag=f"cgB{rep}")
                mgA = accp.tile([128, CHUNKS], F32, tag=f"mgA{rep}")
                mgB = accp.tile([128, CHUNKS], F32, tag=f"mgB{rep}")
                for r in range(CHUNKS):
                    ra, rb = r * 128, (r + 1) * 128
                    for g in range(GROUPS):
                        p2 = ps.tile([128, GCOLS], F32, tag="pp")
                        for q in range(GCOLS // 512):
                            c0 = g * GCOLS + q * 512
                            qs = slice(q * 512, (q + 1) * 512)
                            nc.tensor.matmul(p2[:, qs], a0t[:, ra:rb],
                                             b0t[:, c0:c0 + 512],
                                             start=True, stop=False)
                            nc.tensor.matmul(p2[:, qs], a1t[:, ra:rb],
                                             b1t[:, c0:c0 + 512],
                                             start=False, stop=False)
                            nc.tensor.matmul(p2[:, qs], l2t[:, ra:rb],
                                             r2t[:, c0:c0 + 512],
                                             start=False, stop=True)
                        cmp = wrk.tile([128, GCOLS], BF16, tag="cmp")
                        nc.vector.tensor_scalar(
                            cmp[:], p2[:], dsq[:, r:r + 1], 0.0,
                            op0=mybir.AluOpType.is_lt,
                            op1=mybir.AluOpType.add,
                            accum_out=(cgA if g == 0 else cgB)[:, r:r + 1])
                        mn = wrk.tile([128, GCOLS], F32, tag="mn")
                        nc.vector.tensor_scalar(
                            mn[:], p2[:], dsq[:, r:r + 1], None,
                            op0=mybir.AluOpType.min)
                        msq = wrk.tile([128, GCOLS], BF16, tag="msq")
                        nc.scalar.activation(
                            msq[:], mn[:], mybir.ActivationFunctionType.Sqrt,
                            accum_out=(mgA if g == 0 else mgB)[:, r:r + 1])

                c32 = accp.tile([128, CHUNKS], F32, tag=f"c32{rep}")
                m32 = accp.tile([128, CHUNKS], F32, tag=f"m32{rep}")
                nc.vector.tensor_tensor(c32[:], cgA[:], cgB[:],
                                        op=mybir.AluOpType.add)
                nc.vector.tensor_tensor(m32[:], mgA[:], mgB[:],
                                        op=mybir.AluOpType.add)

                if rep == replicas - 1:
                    nc.sync.dma_start(dneg_o[:], dneg[:])
                    nc.sync.dma_start(c_o[:], c32[:])
                    nc.sync.dma_start(m_o[:], m32[:])
                    nc.sync.dma_start(rs_o[:], rsar[:])
    nc.compile()
    return nc


def get_nc(replicas: int = 1):
    key = ("nc", replicas)
    if key not in _CACHE:
        _CACHE[key] = _build(replicas)
    return _CACHE[key]


def _prep(inputs: np.ndarray, targets: np.ndarray):
    """Host-side exact preprocessing. Returns per-core input maps + host state."""
    x = np.asarray(inputs, np.float32)
    t = np.asarray(targets).astype(np.int64)

    counts = np.bincount(t, minlength=ID).astype(np.float64)
    if counts.min() > 0:
        order = np.argsort(t, kind="stable")
        bnd = np.searchsorted(t[order], np.arange(ID))
        sums = np.add.reduceat(x[order].astype(np.float64), bnd, axis=0)
    else:
        sums = np.zeros((ID, D), np.float64)
        np.add.at(sums, t, x.astype(np.float64))
    centers64 = sums / counts[:, None]
    centers = centers64.astype(np.float32)

    cid = t[np.arange(ID) * NUM_POS]           # id each row's mask selects
    cn = (centers.astype(np.float64) ** 2).sum(1)          # [ID]
    xn = (x.astype(np.float64) ** 2).sum(1)                # [N]

    # positive pairs (i=row, j=sample with t_j == cid[i]); exact in f64
    if np.array_equal(cid, np.arange(ID)):
        pos_row = t                             # row index for sample j
        pos_j = np.arange(N)
    else:  # general fallback
        order = np.argsort(t, kind="stable")
        bnd = np.searchsorted(t[order], np.arange(ID + 1))
        rows, js = [], []
        for i in range(ID):
            sel = order[bnd[cid[i]]:bnd[cid[i] + 1]]
            rows.append(np.full(len(sel), i)); js.append(sel)
        pos_row = np.concatenate(rows); pos_j = np.concatenate(js)
    diff = x[pos_j].astype(np.float64) - centers64[pos_row]
    pos_d = np.sqrt((diff ** 2).sum(1))         # [npairs]

    valid_pos = pos_d > EPS
    ap_mean = pos_d[valid_pos].sum() / max(valid_pos.sum(), 1)

    possum_row = np.bincount(pos_row, weights=pos_d, minlength=ID)
    nneg_row = N - counts[cid]                  # valid negatives per row
    row_cnt = counts[cid]                       # positive entries per row

    # device inputs
    A = (-2.0 * centers.T).astype(ml_dtypes.bfloat16)       # [D, ID]
    lhs2_np = np.stack([cn.astype(np.float32),
                        np.ones(ID, np.float32)])            # [2, ID]
    pos_t = possum_row.astype(np.float32).reshape(CHUNKS, 128).T.copy()
    inv_t = (1.0 / nneg_row).astype(np.float32).reshape(CHUNKS, 128).T.copy()

    in_maps = []
    for c in range(CORES):
        sl = slice(c * J, (c + 1) * J)
        B = np.ascontiguousarray(x[sl].T).astype(ml_dtypes.bfloat16)  # [D, J]
        rhs2_np = np.stack([np.ones(J, np.float32),
                            xn[sl].astype(np.float32)])               # [2, J]
        in_maps.append({
            "a0": np.ascontiguousarray(A[:128]),
            "a1": np.ascontiguousarray(A[128:]),
            "b0": np.ascontiguousarray(B[:128]),
            "b1": np.ascontiguousarray(B[128:]),
            "lhs2": lhs2_np,
            "rhs2": rhs2_np,
            "possum": pos_t,
            "invn": inv_t,
        })
    host = dict(pos_row=pos_row, pos_d=pos_d, ap_mean=ap_mean,
                row_cnt=row_cnt, nneg_row=nneg_row)
    return in_maps, host


def _finish(results, host):
    def vec(a):  # [128, CHUNKS] -> [ID] with id = chunk*128 + p
        return np.asarray(a, np.float64).T.ravel()

    dneg = vec(results[0]["dneg"])
    C = sum(vec(r["c32"]) for r in results)
    M = sum(vec(r["m32"]) for r in results)

    pos_row, pos_d = host["pos_row"], host["pos_d"]
    under = pos_d < dneg[pos_row]
    poscnt_under = np.bincount(pos_row, weights=under.astype(np.float64),
                               minlength=ID)
    possum_under = np.bincount(pos_row, weights=pos_d * under, minlength=ID)

    S_hard = M - dneg * (N - C) - possum_under
    C_hard = C - poscnt_under
    row_an = S_hard / np.maximum(C_hard, 1.0)
    an_mean = row_an.mean()
    return np.float32(host["ap_mean"] / an_mean)


def kernel(inputs: np.ndarray, targets: np.ndarray) -> np.ndarray:
    in_maps, host = _prep(inputs, targets)
    nc = get_nc()
    res = run_bass_kernel_spmd(nc, in_maps, list(range(CORES)))
    return _finish(res.results, host)


if __name__ == "__main__":
    rng = np.random.default_rng(0)
    x = rng.standard_normal((N, D)).astype(np.float32)
    t = (np.arange(N) // NUM_POS) % ID
    print(kernel(x, t))
